# revision 5
# baseline (speedup 1.0000x reference)
"""ChiENN layer Trainium2 kernel (8-core data-parallel SPMD).

Math (valid for inputs with no -1 padding in circle_index, which the input
spec guarantees: randint in [0, N)):

    num_neighbors == 4 for every node, all masks all-true, and the K=3
    window sum never crosses a node's circle row. Hence per node n:

      h_c   = sum_{i<3} x[ci[n, c+i]] @ Wk[i] + B3          (c = 0..3)
      msg_c = elu(h_c) @ Wf + bf
      agg   = sum_c msg_c + x[n]@Ws + bs + x[pni[n]]@Wp + bp
      out   = elu(agg)

    with B3 = bk[0]+bk[1]+bk[2].  Using elu(z) = relu(z) + min(exp(z)-1, 0)
    both elu terms feed plain Wf matmuls, so the constant biases collapse to
    cb = 4*bf + bs + bp applied once pre-final-elu.

Per-core layout: nodes sharded contiguously, 12800 per core (N padded to
102400), processed in 25 blocks of 512 nodes (4 subtiles of 128).  All
gathers (6 circle + 1 parallel per node) are indirect DMAs of 512B rows from
the full x kept in DRAM; gathered tiles are PE-transposed into [D, nodes]
layout so every matmul contracts over D on the partition axis.  The output is
produced transposed ([D, nodes]) and un-transposed on the host.
"""

import numpy as np

import concourse.bass as bass
import concourse.mybir as mybir
import concourse.tile as tile
from concourse import bacc
from concourse.bass import IndirectOffsetOnAxis
from concourse.bass_utils import run_bass_kernel_spmd
from concourse.masks import make_identity

N, D, C, K = 100000, 128, 6, 3
NCORES = 8
BLK = 512                 # nodes per block
SUB = BLK // 128          # subtiles per block
NB = 25                   # blocks per core
NS = BLK * NB             # nodes per core (12800)
NPAD = NS * NCORES        # padded node count (102400)
NSLOT = C + 2             # 6 circle + 1 parallel + 1 self gather

F32 = mybir.dt.float32
F32R = mybir.dt.float32r
I32 = mybir.dt.int32

AF = mybir.ActivationFunctionType
OP = mybir.AluOpType


def build_module(nb: int = NB):
    """Build + compile the per-core Bass program (same program on all cores)."""
    nc = bacc.Bacc("TRN2", target_bir_lowering=False, debug=False,
                   num_devices=NCORES)

    ns = nb * BLK
    xg = nc.dram_tensor("xg", [N, D], F32, kind="ExternalInput").ap()
    idxg = nc.dram_tensor("idxg", [nb, 128, NSLOT * SUB], I32,
                          kind="ExternalInput").ap()
    wdram = {
        name: nc.dram_tensor(name, [D, D], F32, kind="ExternalInput").ap()
        for name in ("wk0", "wk1", "wk2", "wf", "ws", "wp")
    }
    b3d = nc.dram_tensor("b3", [D, 1], F32, kind="ExternalInput").ap()
    cbd = nc.dram_tensor("cb", [D, 1], F32, kind="ExternalInput").ap()
    yt = nc.dram_tensor("yt", [D, ns], F32, kind="ExternalOutput").ap()

    with tile.TileContext(nc) as tc:
        with (
            tc.tile_pool(name="const", bufs=1) as constp,
            tc.tile_pool(name="gp", bufs=2) as gp,
            tc.tile_pool(name="gtp", bufs=2 * NSLOT) as gtp,
            tc.tile_pool(name="actp", bufs=3) as actp,
            tc.tile_pool(name="outp", bufs=3) as outp,
            tc.tile_pool(name="idxp", bufs=3) as idxp,
            tc.tile_pool(name="pst", bufs=2, space="PSUM") as pst,
            tc.tile_pool(name="psh", bufs=4, space="PSUM") as psh,
            tc.tile_pool(name="psm", bufs=2, space="PSUM") as psm,
        ):
            # ---- constants ----
            wsb = {}
            for name in ("wk0", "wk1", "wk2", "wf", "ws", "wp"):
                w_raw = constp.tile([D, D], F32, name=f"{name}_raw", tag=f"{name}_raw")
                nc.sync.dma_start(out=w_raw[:], in_=wdram[name][:])
                w_t = constp.tile([D, D], F32R, name=f"{name}_s")
                nc.vector.tensor_copy(out=w_t[:], in_=w_raw[:])
                wsb[name] = w_t
            b3_t = constp.tile([D, 1], F32, name="b3_s")
            nc.sync.dma_start(out=b3_t[:], in_=b3d[:])
            cb_t = constp.tile([D, 1], F32, name="cb_s")
            nc.sync.dma_start(out=cb_t[:], in_=cbd[:])
            ident = constp.tile([D, D], F32, name="ident")
            make_identity(nc, ident[:])

            wk_s = [wsb["wk0"], wsb["wk1"], wsb["wk2"]]

            for b in range(nb):
                # ---- loads ----
                idx_t = idxp.tile([128, NSLOT * SUB], I32, name="idx_t")
                nc.sync.dma_start(out=idx_t[:], in_=idxg[b])

                # gathered rows: g[p, s*D:(s+1)*D] = x[idx[p, s]], s = j*SUB+t.
                # HW indirect DMA consumes ONE index per partition per
                # instruction, so issue one gather per (slot, subtile).
                g = gp.tile([128, NSLOT * SUB * D], F32, name="g")
                for s in range(NSLOT * SUB):
                    nc.gpsimd.indirect_dma_start(
                        out=g[:, s * D:(s + 1) * D],
                        out_offset=None,
                        in_=xg[:],
                        in_offset=IndirectOffsetOnAxis(ap=idx_t[:, s:s + 1],
                                                       axis=0),
                    )

                # ---- transpose each gathered [128 nodes, D] tile to [D, nodes] ----
                gts = []
                for j in range(NSLOT):
                    ps_t = pst.tile([D, BLK], F32, name="ps_t", tag="ps_t")
                    for t in range(SUB):
                        s = j * SUB + t
                        nc.tensor.transpose(
                            ps_t[:, t * 128:(t + 1) * 128],
                            g[:, s * D:(s + 1) * D],
                            ident[:],
                        )
                    gt_j = gtp.tile([D, BLK], F32R, name=f"gt{j}", tag="gt")
                    if j < 3:
                        nc.scalar.copy(out=gt_j[:], in_=ps_t[:])
                    else:
                        nc.vector.tensor_copy(out=gt_j[:], in_=ps_t[:])
                    gts.append(gt_j)

                # ---- window matmuls: h_c = sum_i Wk[i].T @ gt_{c+i} ----
                hs = [psh.tile([D, BLK], F32, name=f"h{c}", tag="h")
                      for c in range(4)]
                for i in range(K):
                    for c in range(4):
                        nc.tensor.matmul(
                            hs[c][:],
                            lhsT=wk_s[i][:],
                            rhs=gts[c + i][:],
                            start=(i == 0),
                            stop=(i == K - 1),
                        )

                # ---- aggregate bank: m = self + parallel + sum_c elu(h_c)@Wf ----
                m = psm.tile([D, BLK], F32, name="m", tag="m")
                nc.tensor.matmul(m[:], lhsT=wsb["ws"][:],
                                 rhs=gts[7][:],
                                 start=True, stop=False)
                nc.tensor.matmul(m[:], lhsT=wsb["wp"][:],
                                 rhs=gts[6][:],
                                 start=False, stop=False)

                for c in range(4):
                    v_c = actp.tile([D, BLK], F32R, name="v_c", tag="v")
                    if c < 2:
                        nc.scalar.activation(v_c[:], hs[c][:], AF.Relu,
                                             bias=b3_t[:, :1])
                    else:
                        nc.vector.tensor_scalar(
                            out=v_c[:], in0=hs[c][:], scalar1=b3_t[:, :1],
                            scalar2=0.0, op0=OP.add, op1=OP.max)
                    e_c = actp.tile([D, BLK], F32, name="e_c", tag="e")
                    nc.scalar.activation(e_c[:], hs[c][:], AF.Exp,
                                         bias=b3_t[:, :1])
                    u_c = actp.tile([D, BLK], F32R, name="u_c", tag="u")
                    nc.vector.tensor_scalar(
                        out=u_c[:], in0=e_c[:], scalar1=1.0, scalar2=0.0,
                        op0=OP.subtract, op1=OP.min)
                    nc.tensor.matmul(m[:], lhsT=wsb["wf"][:],
                                     rhs=v_c[:],
                                     start=False, stop=False)
                    nc.tensor.matmul(m[:], lhsT=wsb["wf"][:],
                                     rhs=u_c[:],
                                     start=False, stop=(c == 3))

                # ---- final elu(m + cb) ----
                v_f = actp.tile([D, BLK], F32, name="v_f", tag="v")
                nc.scalar.activation(v_f[:], m[:], AF.Relu, bias=cb_t[:, :1])
                e_f = actp.tile([D, BLK], F32, name="e_f", tag="e")
                nc.scalar.activation(e_f[:], m[:], AF.Exp, bias=cb_t[:, :1])
                u_f = actp.tile([D, BLK], F32, name="u_f", tag="u")
                nc.vector.tensor_scalar(
                    out=u_f[:], in0=e_f[:], scalar1=1.0, scalar2=0.0,
                    op0=OP.subtract, op1=OP.min)
                o_t = outp.tile([D, BLK], F32, name="o_t", tag="o")
                nc.vector.tensor_tensor(out=o_t[:], in0=v_f[:], in1=u_f[:],
                                        op=OP.add)
                nc.sync.dma_start(out=yt[:, b * BLK:(b + 1) * BLK], in_=o_t[:])

    nc.compile()
    return nc


def build_in_maps(inputs: dict, nb: int = NB):
    """Shard/arrange FULL inputs into 8 per-core input maps."""
    x = np.ascontiguousarray(np.asarray(inputs["x"], dtype=np.float32))
    ci = np.asarray(inputs["circle_index"], dtype=np.int32)
    pni = np.asarray(inputs["parallel_node_index"], dtype=np.int32)
    Wk = np.asarray(inputs["Wk"], dtype=np.float32)
    bk = np.asarray(inputs["bk"], dtype=np.float32)
    Wf = np.ascontiguousarray(np.asarray(inputs["Wf"], dtype=np.float32))
    bf = np.asarray(inputs["bf"], dtype=np.float32)
    Ws = np.ascontiguousarray(np.asarray(inputs["Ws"], dtype=np.float32))
    bs = np.asarray(inputs["bs"], dtype=np.float32)
    Wp = np.ascontiguousarray(np.asarray(inputs["Wp"], dtype=np.float32))
    bp = np.asarray(inputs["bp"], dtype=np.float32)

    ns = nb * BLK
    npad = ns * NCORES

    self_idx = np.arange(N, dtype=np.int32)[:, None]
    idx_all = np.concatenate([ci, pni[:, None], self_idx], axis=1)  # [N, 8]
    idx_all = np.clip(idx_all, 0, N - 1)
    idx_pad = np.zeros((npad, NSLOT), np.int32)
    idx_pad[:N] = idx_all[: min(N, npad)]

    b3 = (bk[0] + bk[1] + bk[2]).reshape(D, 1).astype(np.float32)
    cb = (4.0 * bf + bs + bp).reshape(D, 1).astype(np.float32)

    common = {
        "xg": x,
        "wk0": np.ascontiguousarray(Wk[0]),
        "wk1": np.ascontiguousarray(Wk[1]),
        "wk2": np.ascontiguousarray(Wk[2]),
        "wf": Wf, "ws": Ws, "wp": Wp,
        "b3": b3, "cb": cb,
    }
    in_maps = []
    for c in range(NCORES):
        sl = idx_pad[c * ns:(c + 1) * ns]                     # [ns, NSLOT]
        # -> [nb, 128(p), NSLOT(j)*SUB(t)] with slot s = j*SUB + t
        idxc = (sl.reshape(nb, SUB, 128, NSLOT)
                  .transpose(0, 2, 3, 1)
                  .reshape(nb, 128, NSLOT * SUB))
        in_maps.append({
            **common,
            "idxg": np.ascontiguousarray(idxc),
        })
    return in_maps


def assemble_output(results, nb: int = NB):
    ns = nb * BLK
    out = np.empty((NCORES * ns, D), np.float32)
    for c in range(NCORES):
        out[c * ns:(c + 1) * ns] = results[c]["yt"].T
    return np.ascontiguousarray(out[:N])


_NC_CACHE = {}


def kernel(**inputs) -> np.ndarray:
    if "nc" not in _NC_CACHE:
        _NC_CACHE["nc"] = build_module()
    nc = _NC_CACHE["nc"]
    in_maps = build_in_maps(inputs)
    res = run_bass_kernel_spmd(nc, in_maps, core_ids=list(range(NCORES)))
    return assemble_output(res.results)


# revision 6
# speedup vs baseline: 1.0122x; 1.0122x over previous
"""ChiENN layer Trainium2 kernel (8-core data-parallel SPMD).

Math (valid for inputs with no -1 padding in circle_index, which the input
spec guarantees: randint in [0, N)):

    num_neighbors == 4 for every node, all masks all-true, and the K=3
    window sum never crosses a node's circle row. Hence per node n:

      h_c   = sum_{i<3} x[ci[n, c+i]] @ Wk[i] + B3          (c = 0..3)
      msg_c = elu(h_c) @ Wf + bf
      agg   = sum_c msg_c + x[n]@Ws + bs + x[pni[n]]@Wp + bp
      out   = elu(agg)

    with B3 = bk[0]+bk[1]+bk[2].  Using elu(z) = relu(z) + min(exp(z)-1, 0)
    both elu terms feed plain Wf matmuls, so the constant biases collapse to
    cb = 4*bf + bs + bp applied once pre-final-elu.

Per-core layout: nodes sharded contiguously, 12800 per core (N padded to
102400), processed in 25 blocks of 512 nodes (4 subtiles of 128).  All
gathers (6 circle + 1 parallel per node) are indirect DMAs of 512B rows from
the full x kept in DRAM; gathered tiles are PE-transposed into [D, nodes]
layout so every matmul contracts over D on the partition axis.  The output is
produced transposed ([D, nodes]) and un-transposed on the host.
"""

import numpy as np

import concourse.bass as bass
import concourse.mybir as mybir
import concourse.tile as tile
from concourse import bacc
from concourse.bass import IndirectOffsetOnAxis
from concourse.bass_utils import run_bass_kernel_spmd
from concourse.masks import make_identity

N, D, C, K = 100000, 128, 6, 3
NCORES = 8
BLK = 512                 # nodes per block
SUB = BLK // 128          # subtiles per block
NB = 25                   # blocks per core
NS = BLK * NB             # nodes per core (12800)
NPAD = NS * NCORES        # padded node count (102400)
NSLOT = C + 2             # 6 circle + 1 parallel + 1 self gather

F32 = mybir.dt.float32
F32R = mybir.dt.float32r
I32 = mybir.dt.int32

AF = mybir.ActivationFunctionType
OP = mybir.AluOpType


def build_module(nb: int = NB):
    """Build + compile the per-core Bass program (same program on all cores)."""
    nc = bacc.Bacc("TRN2", target_bir_lowering=False, debug=False,
                   num_devices=NCORES, num_swdge_queues=4)

    ns = nb * BLK
    xg = nc.dram_tensor("xg", [N, D], F32, kind="ExternalInput").ap()
    idxg = nc.dram_tensor("idxg", [nb, 128, NSLOT * SUB], I32,
                          kind="ExternalInput").ap()
    wdram = {
        name: nc.dram_tensor(name, [D, D], F32, kind="ExternalInput").ap()
        for name in ("wk0", "wk1", "wk2", "wf", "ws", "wp")
    }
    b3d = nc.dram_tensor("b3", [D, 1], F32, kind="ExternalInput").ap()
    cbd = nc.dram_tensor("cb", [D, 1], F32, kind="ExternalInput").ap()
    yt = nc.dram_tensor("yt", [D, ns], F32, kind="ExternalOutput").ap()

    with tile.TileContext(nc) as tc:
        with (
            tc.tile_pool(name="const", bufs=1) as constp,
            tc.tile_pool(name="gp", bufs=2) as gp,
            tc.tile_pool(name="gtp", bufs=2 * NSLOT) as gtp,
            tc.tile_pool(name="actp", bufs=3) as actp,
            tc.tile_pool(name="outp", bufs=3) as outp,
            tc.tile_pool(name="idxp", bufs=3) as idxp,
            tc.tile_pool(name="pst", bufs=2, space="PSUM") as pst,
            tc.tile_pool(name="psh", bufs=4, space="PSUM") as psh,
            tc.tile_pool(name="psm", bufs=2, space="PSUM") as psm,
        ):
            # ---- constants ----
            wsb = {}
            for name in ("wk0", "wk1", "wk2", "wf", "ws", "wp"):
                w_raw = constp.tile([D, D], F32, name=f"{name}_raw", tag=f"{name}_raw")
                nc.sync.dma_start(out=w_raw[:], in_=wdram[name][:])
                w_t = constp.tile([D, D], F32R, name=f"{name}_s")
                nc.vector.tensor_copy(out=w_t[:], in_=w_raw[:])
                wsb[name] = w_t
            b3_t = constp.tile([D, 1], F32, name="b3_s")
            nc.sync.dma_start(out=b3_t[:], in_=b3d[:])
            cb_t = constp.tile([D, 1], F32, name="cb_s")
            nc.sync.dma_start(out=cb_t[:], in_=cbd[:])
            ident = constp.tile([D, D], F32, name="ident")
            make_identity(nc, ident[:])

            wk_s = [wsb["wk0"], wsb["wk1"], wsb["wk2"]]

            for b in range(nb):
                # ---- loads ----
                idx_t = idxp.tile([128, NSLOT * SUB], I32, name="idx_t")
                nc.sync.dma_start(out=idx_t[:], in_=idxg[b])

                # gathered rows: g[p, s*D:(s+1)*D] = x[idx[p, s]], s = j*SUB+t.
                # HW indirect DMA consumes ONE index per partition per
                # instruction, so issue one gather per (slot, subtile).
                g = gp.tile([128, NSLOT * SUB * D], F32, name="g")
                for s in range(NSLOT * SUB):
                    gi = nc.gpsimd.indirect_dma_start(
                        out=g[:, s * D:(s + 1) * D],
                        out_offset=None,
                        in_=xg[:],
                        in_offset=IndirectOffsetOnAxis(ap=idx_t[:, s:s + 1],
                                                       axis=0),
                    )
                    q = s % 4
                    if q:
                        gi.ins.queue = f"qPoolDynamic{q}"

                # ---- transpose each gathered [128 nodes, D] tile to [D, nodes] ----
                gts = []
                for j in range(NSLOT):
                    ps_t = pst.tile([D, BLK], F32, name="ps_t", tag="ps_t")
                    for t in range(SUB):
                        s = j * SUB + t
                        nc.tensor.transpose(
                            ps_t[:, t * 128:(t + 1) * 128],
                            g[:, s * D:(s + 1) * D],
                            ident[:],
                        )
                    gt_j = gtp.tile([D, BLK], F32R, name=f"gt{j}", tag="gt")
                    if j < 3:
                        nc.scalar.copy(out=gt_j[:], in_=ps_t[:])
                    else:
                        nc.vector.tensor_copy(out=gt_j[:], in_=ps_t[:])
                    gts.append(gt_j)

                # ---- window matmuls: h_c = sum_i Wk[i].T @ gt_{c+i} ----
                hs = [psh.tile([D, BLK], F32, name=f"h{c}", tag="h")
                      for c in range(4)]
                for i in range(K):
                    for c in range(4):
                        nc.tensor.matmul(
                            hs[c][:],
                            lhsT=wk_s[i][:],
                            rhs=gts[c + i][:],
                            start=(i == 0),
                            stop=(i == K - 1),
                        )

                # ---- aggregate bank: m = self + parallel + sum_c elu(h_c)@Wf ----
                m = psm.tile([D, BLK], F32, name="m", tag="m")
                nc.tensor.matmul(m[:], lhsT=wsb["ws"][:],
                                 rhs=gts[7][:],
                                 start=True, stop=False)
                nc.tensor.matmul(m[:], lhsT=wsb["wp"][:],
                                 rhs=gts[6][:],
                                 start=False, stop=False)

                for c in range(4):
                    v_c = actp.tile([D, BLK], F32R, name="v_c", tag="v")
                    if c < 2:
                        nc.scalar.activation(v_c[:], hs[c][:], AF.Relu,
                                             bias=b3_t[:, :1])
                    else:
                        nc.vector.tensor_scalar(
                            out=v_c[:], in0=hs[c][:], scalar1=b3_t[:, :1],
                            scalar2=0.0, op0=OP.add, op1=OP.max)
                    e_c = actp.tile([D, BLK], F32, name="e_c", tag="e")
                    nc.scalar.activation(e_c[:], hs[c][:], AF.Exp,
                                         bias=b3_t[:, :1])
                    u_c = actp.tile([D, BLK], F32R, name="u_c", tag="u")
                    nc.vector.tensor_scalar(
                        out=u_c[:], in0=e_c[:], scalar1=1.0, scalar2=0.0,
                        op0=OP.subtract, op1=OP.min)
                    nc.tensor.matmul(m[:], lhsT=wsb["wf"][:],
                                     rhs=v_c[:],
                                     start=False, stop=False)
                    nc.tensor.matmul(m[:], lhsT=wsb["wf"][:],
                                     rhs=u_c[:],
                                     start=False, stop=(c == 3))

                # ---- final elu(m + cb) ----
                v_f = actp.tile([D, BLK], F32, name="v_f", tag="v")
                nc.scalar.activation(v_f[:], m[:], AF.Relu, bias=cb_t[:, :1])
                e_f = actp.tile([D, BLK], F32, name="e_f", tag="e")
                nc.scalar.activation(e_f[:], m[:], AF.Exp, bias=cb_t[:, :1])
                u_f = actp.tile([D, BLK], F32, name="u_f", tag="u")
                nc.vector.tensor_scalar(
                    out=u_f[:], in0=e_f[:], scalar1=1.0, scalar2=0.0,
                    op0=OP.subtract, op1=OP.min)
                o_t = outp.tile([D, BLK], F32, name="o_t", tag="o")
                nc.vector.tensor_tensor(out=o_t[:], in0=v_f[:], in1=u_f[:],
                                        op=OP.add)
                nc.sync.dma_start(out=yt[:, b * BLK:(b + 1) * BLK], in_=o_t[:])

    nc.compile()
    return nc


def build_in_maps(inputs: dict, nb: int = NB):
    """Shard/arrange FULL inputs into 8 per-core input maps."""
    x = np.ascontiguousarray(np.asarray(inputs["x"], dtype=np.float32))
    ci = np.asarray(inputs["circle_index"], dtype=np.int32)
    pni = np.asarray(inputs["parallel_node_index"], dtype=np.int32)
    Wk = np.asarray(inputs["Wk"], dtype=np.float32)
    bk = np.asarray(inputs["bk"], dtype=np.float32)
    Wf = np.ascontiguousarray(np.asarray(inputs["Wf"], dtype=np.float32))
    bf = np.asarray(inputs["bf"], dtype=np.float32)
    Ws = np.ascontiguousarray(np.asarray(inputs["Ws"], dtype=np.float32))
    bs = np.asarray(inputs["bs"], dtype=np.float32)
    Wp = np.ascontiguousarray(np.asarray(inputs["Wp"], dtype=np.float32))
    bp = np.asarray(inputs["bp"], dtype=np.float32)

    ns = nb * BLK
    npad = ns * NCORES

    self_idx = np.arange(N, dtype=np.int32)[:, None]
    idx_all = np.concatenate([ci, pni[:, None], self_idx], axis=1)  # [N, 8]
    idx_all = np.clip(idx_all, 0, N - 1)
    idx_pad = np.zeros((npad, NSLOT), np.int32)
    idx_pad[:N] = idx_all[: min(N, npad)]

    b3 = (bk[0] + bk[1] + bk[2]).reshape(D, 1).astype(np.float32)
    cb = (4.0 * bf + bs + bp).reshape(D, 1).astype(np.float32)

    common = {
        "xg": x,
        "wk0": np.ascontiguousarray(Wk[0]),
        "wk1": np.ascontiguousarray(Wk[1]),
        "wk2": np.ascontiguousarray(Wk[2]),
        "wf": Wf, "ws": Ws, "wp": Wp,
        "b3": b3, "cb": cb,
    }
    in_maps = []
    for c in range(NCORES):
        sl = idx_pad[c * ns:(c + 1) * ns]                     # [ns, NSLOT]
        # -> [nb, 128(p), NSLOT(j)*SUB(t)] with slot s = j*SUB + t
        idxc = (sl.reshape(nb, SUB, 128, NSLOT)
                  .transpose(0, 2, 3, 1)
                  .reshape(nb, 128, NSLOT * SUB))
        in_maps.append({
            **common,
            "idxg": np.ascontiguousarray(idxc),
        })
    return in_maps


def assemble_output(results, nb: int = NB):
    ns = nb * BLK
    out = np.empty((NCORES * ns, D), np.float32)
    for c in range(NCORES):
        out[c * ns:(c + 1) * ns] = results[c]["yt"].T
    return np.ascontiguousarray(out[:N])


_NC_CACHE = {}


def kernel(**inputs) -> np.ndarray:
    if "nc" not in _NC_CACHE:
        _NC_CACHE["nc"] = build_module()
    nc = _NC_CACHE["nc"]
    in_maps = build_in_maps(inputs)
    res = run_bass_kernel_spmd(nc, in_maps, core_ids=list(range(NCORES)))
    return assemble_output(res.results)


# revision 7
# speedup vs baseline: 1.1435x; 1.1297x over previous
"""ChiENN layer Trainium2 kernel (8-core data-parallel SPMD).

Math (valid for inputs with no -1 padding in circle_index, which the input
spec guarantees: randint in [0, N)):

    num_neighbors == 4 for every node, all masks all-true, and the K=3
    window sum never crosses a node's circle row. Hence per node n:

      h_c   = sum_{i<3} x[ci[n, c+i]] @ Wk[i] + B3          (c = 0..3)
      msg_c = elu(h_c) @ Wf + bf
      agg   = sum_c msg_c + x[n]@Ws + bs + x[pni[n]]@Wp + bp
      out   = elu(agg)

    with B3 = bk[0]+bk[1]+bk[2].  Using elu(z) = relu(z) + min(exp(z)-1, 0)
    both elu terms feed plain Wf matmuls, so the constant biases collapse to
    cb = 4*bf + bs + bp applied once pre-final-elu.

Per-core layout: nodes sharded contiguously, 12800 per core (N padded to
102400), processed in 25 blocks of 512 nodes (4 subtiles of 128).  All
gathers (6 circle + 1 parallel per node) are indirect DMAs of 512B rows from
the full x kept in DRAM; gathered tiles are PE-transposed into [D, nodes]
layout so every matmul contracts over D on the partition axis.  The output is
produced transposed ([D, nodes]) and un-transposed on the host.
"""

import numpy as np

import concourse.bass as bass
import concourse.mybir as mybir
import concourse.tile as tile
from concourse import bacc
from concourse.bass import IndirectOffsetOnAxis
from concourse.bass_utils import run_bass_kernel_spmd
from concourse.masks import make_identity

N, D, C, K = 100000, 128, 6, 3
NCORES = 8
BLK = 512                 # nodes per block
SUB = BLK // 128          # subtiles per block
NB = 25                   # blocks per core
NS = BLK * NB             # nodes per core (12800)
NPAD = NS * NCORES        # padded node count (102400)
NSLOT = C + 1             # 6 circle + 1 parallel gather (self is a dense load)

F32 = mybir.dt.float32
F32R = mybir.dt.float32r
I32 = mybir.dt.int32

AF = mybir.ActivationFunctionType
OP = mybir.AluOpType


def build_module(nb: int = NB):
    """Build + compile the per-core Bass program (same program on all cores)."""
    nc = bacc.Bacc("TRN2", target_bir_lowering=False, debug=False,
                   num_devices=NCORES, num_swdge_queues=4)

    ns = nb * BLK
    xg = nc.dram_tensor("xg", [N, D], F32, kind="ExternalInput").ap()
    xts = nc.dram_tensor("xts", [D, ns], F32, kind="ExternalInput").ap()
    idxg = nc.dram_tensor("idxg", [nb, 128, NSLOT * SUB], I32,
                          kind="ExternalInput").ap()
    wdram = {
        name: nc.dram_tensor(name, [D, D], F32, kind="ExternalInput").ap()
        for name in ("wk0", "wk1", "wk2", "wf", "ws", "wp")
    }
    b3d = nc.dram_tensor("b3", [D, 1], F32, kind="ExternalInput").ap()
    cbd = nc.dram_tensor("cb", [D, 1], F32, kind="ExternalInput").ap()
    yt = nc.dram_tensor("yt", [D, ns], F32, kind="ExternalOutput").ap()

    with tile.TileContext(nc) as tc:
        with (
            tc.tile_pool(name="const", bufs=1) as constp,
            tc.tile_pool(name="gp", bufs=2) as gp,
            tc.tile_pool(name="gtp", bufs=2 * NSLOT) as gtp,
            tc.tile_pool(name="actp", bufs=3) as actp,
            tc.tile_pool(name="outp", bufs=3) as outp,
            tc.tile_pool(name="idxp", bufs=3) as idxp,
            tc.tile_pool(name="xtp", bufs=3) as xtp,
            tc.tile_pool(name="pst", bufs=2, space="PSUM") as pst,
            tc.tile_pool(name="psh", bufs=4, space="PSUM") as psh,
            tc.tile_pool(name="psm", bufs=2, space="PSUM") as psm,
        ):
            # ---- constants ----
            wsb = {}
            for name in ("wk0", "wk1", "wk2", "wf", "ws", "wp"):
                w_raw = constp.tile([D, D], F32, name=f"{name}_raw", tag=f"{name}_raw")
                nc.sync.dma_start(out=w_raw[:], in_=wdram[name][:])
                w_t = constp.tile([D, D], F32R, name=f"{name}_s")
                nc.vector.tensor_copy(out=w_t[:], in_=w_raw[:])
                wsb[name] = w_t
            b3_t = constp.tile([D, 1], F32, name="b3_s")
            nc.sync.dma_start(out=b3_t[:], in_=b3d[:])
            cb_t = constp.tile([D, 1], F32, name="cb_s")
            nc.sync.dma_start(out=cb_t[:], in_=cbd[:])
            ident = constp.tile([D, D], F32, name="ident")
            make_identity(nc, ident[:])

            wk_s = [wsb["wk0"], wsb["wk1"], wsb["wk2"]]

            for b in range(nb):
                # ---- loads ----
                idx_t = idxp.tile([128, NSLOT * SUB], I32, name="idx_t")
                nc.sync.dma_start(out=idx_t[:], in_=idxg[b])
                xt_t = xtp.tile([D, BLK], F32, name="xt_t", tag="xt")
                nc.sync.dma_start(out=xt_t[:], in_=xts[:, b * BLK:(b + 1) * BLK])
                xt_r = xtp.tile([D, BLK], F32R, name="xt_r", tag="xtr")
                nc.scalar.copy(out=xt_r[:], in_=xt_t[:])

                # gathered rows: g[p, s*D:(s+1)*D] = x[idx[p, s]], s = j*SUB+t.
                # HW indirect DMA consumes ONE index per partition per
                # instruction, so issue one gather per (slot, subtile).
                g = gp.tile([128, NSLOT * SUB * D], F32, name="g")
                for s in range(NSLOT * SUB):
                    gi = nc.gpsimd.indirect_dma_start(
                        out=g[:, s * D:(s + 1) * D],
                        out_offset=None,
                        in_=xg[:],
                        in_offset=IndirectOffsetOnAxis(ap=idx_t[:, s:s + 1],
                                                       axis=0),
                    )
                    q = s % 4
                    if q:
                        gi.ins.queue = f"qPoolDynamic{q}"

                # ---- transpose each gathered [128 nodes, D] tile to [D, nodes] ----
                gts = []
                for j in range(NSLOT):
                    ps_t = pst.tile([D, BLK], F32, name="ps_t", tag="ps_t")
                    for t in range(SUB):
                        s = j * SUB + t
                        nc.tensor.transpose(
                            ps_t[:, t * 128:(t + 1) * 128],
                            g[:, s * D:(s + 1) * D],
                            ident[:],
                        )
                    gt_j = gtp.tile([D, BLK], F32R, name=f"gt{j}", tag="gt")
                    if j < 3:
                        nc.scalar.copy(out=gt_j[:], in_=ps_t[:])
                    else:
                        nc.vector.tensor_copy(out=gt_j[:], in_=ps_t[:])
                    gts.append(gt_j)

                # ---- window matmuls: h_c = sum_i Wk[i].T @ gt_{c+i} ----
                hs = [psh.tile([D, BLK], F32, name=f"h{c}", tag="h")
                      for c in range(4)]
                for i in range(K):
                    for c in range(4):
                        nc.tensor.matmul(
                            hs[c][:],
                            lhsT=wk_s[i][:],
                            rhs=gts[c + i][:],
                            start=(i == 0),
                            stop=(i == K - 1),
                        )

                # ---- aggregate bank: m = self + parallel + sum_c elu(h_c)@Wf ----
                m = psm.tile([D, BLK], F32, name="m", tag="m")
                nc.tensor.matmul(m[:], lhsT=wsb["ws"][:],
                                 rhs=xt_r[:],
                                 start=True, stop=False)
                nc.tensor.matmul(m[:], lhsT=wsb["wp"][:],
                                 rhs=gts[6][:],
                                 start=False, stop=False)

                for c in range(4):
                    v_c = actp.tile([D, BLK], F32R, name="v_c", tag="v")
                    if c < 2:
                        nc.scalar.activation(v_c[:], hs[c][:], AF.Relu,
                                             bias=b3_t[:, :1])
                    else:
                        nc.vector.tensor_scalar(
                            out=v_c[:], in0=hs[c][:], scalar1=b3_t[:, :1],
                            scalar2=0.0, op0=OP.add, op1=OP.max)
                    e_c = actp.tile([D, BLK], F32, name="e_c", tag="e")
                    nc.scalar.activation(e_c[:], hs[c][:], AF.Exp,
                                         bias=b3_t[:, :1])
                    u_c = actp.tile([D, BLK], F32R, name="u_c", tag="u")
                    nc.vector.tensor_scalar(
                        out=u_c[:], in0=e_c[:], scalar1=1.0, scalar2=0.0,
                        op0=OP.subtract, op1=OP.min)
                    nc.tensor.matmul(m[:], lhsT=wsb["wf"][:],
                                     rhs=v_c[:],
                                     start=False, stop=False)
                    nc.tensor.matmul(m[:], lhsT=wsb["wf"][:],
                                     rhs=u_c[:],
                                     start=False, stop=(c == 3))

                # ---- final elu(m + cb) ----
                v_f = actp.tile([D, BLK], F32, name="v_f", tag="v")
                nc.scalar.activation(v_f[:], m[:], AF.Relu, bias=cb_t[:, :1])
                e_f = actp.tile([D, BLK], F32, name="e_f", tag="e")
                nc.scalar.activation(e_f[:], m[:], AF.Exp, bias=cb_t[:, :1])
                u_f = actp.tile([D, BLK], F32, name="u_f", tag="u")
                nc.vector.tensor_scalar(
                    out=u_f[:], in0=e_f[:], scalar1=1.0, scalar2=0.0,
                    op0=OP.subtract, op1=OP.min)
                o_t = outp.tile([D, BLK], F32, name="o_t", tag="o")
                nc.vector.tensor_tensor(out=o_t[:], in0=v_f[:], in1=u_f[:],
                                        op=OP.add)
                nc.sync.dma_start(out=yt[:, b * BLK:(b + 1) * BLK], in_=o_t[:])

    nc.compile()
    return nc


def build_in_maps(inputs: dict, nb: int = NB):
    """Shard/arrange FULL inputs into 8 per-core input maps."""
    x = np.ascontiguousarray(np.asarray(inputs["x"], dtype=np.float32))
    ci = np.asarray(inputs["circle_index"], dtype=np.int32)
    pni = np.asarray(inputs["parallel_node_index"], dtype=np.int32)
    Wk = np.asarray(inputs["Wk"], dtype=np.float32)
    bk = np.asarray(inputs["bk"], dtype=np.float32)
    Wf = np.ascontiguousarray(np.asarray(inputs["Wf"], dtype=np.float32))
    bf = np.asarray(inputs["bf"], dtype=np.float32)
    Ws = np.ascontiguousarray(np.asarray(inputs["Ws"], dtype=np.float32))
    bs = np.asarray(inputs["bs"], dtype=np.float32)
    Wp = np.ascontiguousarray(np.asarray(inputs["Wp"], dtype=np.float32))
    bp = np.asarray(inputs["bp"], dtype=np.float32)

    ns = nb * BLK
    npad = ns * NCORES

    idx_all = np.concatenate([ci, pni[:, None]], axis=1)            # [N, 7]
    idx_all = np.clip(idx_all, 0, N - 1)
    idx_pad = np.zeros((npad, NSLOT), np.int32)
    idx_pad[:N] = idx_all[: min(N, npad)]

    xt = np.zeros((D, npad), np.float32)
    xt[:, :N] = x.T[:, : min(N, npad)]

    b3 = (bk[0] + bk[1] + bk[2]).reshape(D, 1).astype(np.float32)
    cb = (4.0 * bf + bs + bp).reshape(D, 1).astype(np.float32)

    common = {
        "xg": x,
        "wk0": np.ascontiguousarray(Wk[0]),
        "wk1": np.ascontiguousarray(Wk[1]),
        "wk2": np.ascontiguousarray(Wk[2]),
        "wf": Wf, "ws": Ws, "wp": Wp,
        "b3": b3, "cb": cb,
    }
    in_maps = []
    for c in range(NCORES):
        sl = idx_pad[c * ns:(c + 1) * ns]                     # [ns, NSLOT]
        # -> [nb, 128(p), NSLOT(j)*SUB(t)] with slot s = j*SUB + t
        idxc = (sl.reshape(nb, SUB, 128, NSLOT)
                  .transpose(0, 2, 3, 1)
                  .reshape(nb, 128, NSLOT * SUB))
        in_maps.append({
            **common,
            "idxg": np.ascontiguousarray(idxc),
            "xts": np.ascontiguousarray(xt[:, c * ns:(c + 1) * ns]),
        })
    return in_maps


def assemble_output(results, nb: int = NB):
    ns = nb * BLK
    out = np.empty((NCORES * ns, D), np.float32)
    for c in range(NCORES):
        out[c * ns:(c + 1) * ns] = results[c]["yt"].T
    return np.ascontiguousarray(out[:N])


_NC_CACHE = {}


def kernel(**inputs) -> np.ndarray:
    if "nc" not in _NC_CACHE:
        _NC_CACHE["nc"] = build_module()
    nc = _NC_CACHE["nc"]
    in_maps = build_in_maps(inputs)
    res = run_bass_kernel_spmd(nc, in_maps, core_ids=list(range(NCORES)))
    return assemble_output(res.results)


# revision 8
# speedup vs baseline: 1.1525x; 1.0079x over previous
"""ChiENN layer Trainium2 kernel (8-core data-parallel SPMD).

Math (valid for inputs with no -1 padding in circle_index, which the input
spec guarantees: randint in [0, N)):

    num_neighbors == 4 for every node, all masks all-true, and the K=3
    window sum never crosses a node's circle row. Hence per node n:

      h_c   = sum_{i<3} x[ci[n, c+i]] @ Wk[i] + B3          (c = 0..3)
      msg_c = elu(h_c) @ Wf + bf
      agg   = sum_c msg_c + x[n]@Ws + bs + x[pni[n]]@Wp + bp
      out   = elu(agg)

    with B3 = bk[0]+bk[1]+bk[2].  Using elu(z) = relu(z) + min(exp(z)-1, 0)
    both elu terms feed plain Wf matmuls, so the constant biases collapse to
    cb = 4*bf + bs + bp applied once pre-final-elu.

Per-core layout: nodes sharded contiguously, 12800 per core (N padded to
102400), processed in 25 blocks of 512 nodes (4 subtiles of 128).  All
gathers (6 circle + 1 parallel per node) are indirect DMAs of 512B rows from
the full x kept in DRAM; gathered tiles are PE-transposed into [D, nodes]
layout so every matmul contracts over D on the partition axis.  The output is
produced transposed ([D, nodes]) and un-transposed on the host.
"""

import numpy as np

import concourse.bass as bass
import concourse.mybir as mybir
import concourse.tile as tile
from concourse import bacc
from concourse.bass import IndirectOffsetOnAxis
from concourse.bass_utils import run_bass_kernel_spmd
from concourse.masks import make_identity

N, D, C, K = 100000, 128, 6, 3
NCORES = 8
BLK = 512                 # nodes per block
SUB = BLK // 128          # subtiles per block
NB = 25                   # blocks per core
NS = BLK * NB             # nodes per core (12800)
NPAD = NS * NCORES        # padded node count (102400)
NSLOT = C + 1             # 6 circle + 1 parallel gather (self is a dense load)

F32 = mybir.dt.float32
F32R = mybir.dt.float32r
I32 = mybir.dt.int32

AF = mybir.ActivationFunctionType
OP = mybir.AluOpType


def build_module(nb: int = NB):
    """Build + compile the per-core Bass program (same program on all cores)."""
    nc = bacc.Bacc("TRN2", target_bir_lowering=False, debug=False,
                   num_devices=NCORES, num_swdge_queues=4,
                   dynamic_dma_scratch_size=65536)

    ns = nb * BLK
    xg = nc.dram_tensor("xg", [N, D], F32, kind="ExternalInput").ap()
    xts = nc.dram_tensor("xts", [D, ns], F32, kind="ExternalInput").ap()
    idxg = nc.dram_tensor("idxg", [nb, 128, NSLOT * SUB], I32,
                          kind="ExternalInput").ap()
    wdram = {
        name: nc.dram_tensor(name, [D, D], F32, kind="ExternalInput").ap()
        for name in ("wk0", "wk1", "wk2", "wf", "ws", "wp")
    }
    b3d = nc.dram_tensor("b3", [D, 1], F32, kind="ExternalInput").ap()
    cbd = nc.dram_tensor("cb", [D, 1], F32, kind="ExternalInput").ap()
    yt = nc.dram_tensor("yt", [D, ns], F32, kind="ExternalOutput").ap()

    with tile.TileContext(nc) as tc:
        with (
            tc.tile_pool(name="const", bufs=1) as constp,
            tc.tile_pool(name="gp", bufs=2) as gp,
            tc.tile_pool(name="gtp", bufs=2 * NSLOT) as gtp,
            tc.tile_pool(name="actp", bufs=3) as actp,
            tc.tile_pool(name="outp", bufs=3) as outp,
            tc.tile_pool(name="idxp", bufs=3) as idxp,
            tc.tile_pool(name="xtp", bufs=3) as xtp,
            tc.tile_pool(name="pst", bufs=2, space="PSUM") as pst,
            tc.tile_pool(name="psh", bufs=4, space="PSUM") as psh,
            tc.tile_pool(name="psm", bufs=2, space="PSUM") as psm,
        ):
            # ---- constants ----
            wsb = {}
            for name in ("wk0", "wk1", "wk2", "wf", "ws", "wp"):
                w_raw = constp.tile([D, D], F32, name=f"{name}_raw", tag=f"{name}_raw")
                nc.sync.dma_start(out=w_raw[:], in_=wdram[name][:])
                w_t = constp.tile([D, D], F32R, name=f"{name}_s")
                nc.vector.tensor_copy(out=w_t[:], in_=w_raw[:])
                wsb[name] = w_t
            b3_t = constp.tile([D, 1], F32, name="b3_s")
            nc.sync.dma_start(out=b3_t[:], in_=b3d[:])
            cb_t = constp.tile([D, 1], F32, name="cb_s")
            nc.sync.dma_start(out=cb_t[:], in_=cbd[:])
            ident = constp.tile([D, D], F32, name="ident")
            make_identity(nc, ident[:])

            wk_s = [wsb["wk0"], wsb["wk1"], wsb["wk2"]]

            for b in range(nb):
                # ---- loads ----
                idx_t = idxp.tile([128, NSLOT * SUB], I32, name="idx_t")
                nc.sync.dma_start(out=idx_t[:], in_=idxg[b])
                xt_t = xtp.tile([D, BLK], F32, name="xt_t", tag="xt")
                nc.sync.dma_start(out=xt_t[:], in_=xts[:, b * BLK:(b + 1) * BLK])
                xt_r = xtp.tile([D, BLK], F32R, name="xt_r", tag="xtr")
                nc.scalar.copy(out=xt_r[:], in_=xt_t[:])

                # gathered rows: g[p, s*D:(s+1)*D] = x[idx[p, s]], s = j*SUB+t.
                # HW indirect DMA consumes ONE index per partition per
                # instruction, so issue one gather per (slot, subtile).
                g = gp.tile([128, NSLOT * SUB * D], F32, name="g")
                for s in range(NSLOT * SUB):
                    gi = nc.gpsimd.indirect_dma_start(
                        out=g[:, s * D:(s + 1) * D],
                        out_offset=None,
                        in_=xg[:],
                        in_offset=IndirectOffsetOnAxis(ap=idx_t[:, s:s + 1],
                                                       axis=0),
                    )
                    q = s % 4
                    if q:
                        gi.ins.queue = f"qPoolDynamic{q}"

                # ---- transpose each gathered [128 nodes, D] tile to [D, nodes] ----
                gts = []
                for j in range(NSLOT):
                    ps_t = pst.tile([D, BLK], F32, name="ps_t", tag="ps_t")
                    for t in range(SUB):
                        s = j * SUB + t
                        nc.tensor.transpose(
                            ps_t[:, t * 128:(t + 1) * 128],
                            g[:, s * D:(s + 1) * D],
                            ident[:],
                        )
                    gt_j = gtp.tile([D, BLK], F32R, name=f"gt{j}", tag="gt")
                    if j < 3:
                        nc.scalar.copy(out=gt_j[:], in_=ps_t[:])
                    else:
                        nc.vector.tensor_copy(out=gt_j[:], in_=ps_t[:])
                    gts.append(gt_j)

                # ---- window matmuls: h_c = sum_i Wk[i].T @ gt_{c+i} ----
                hs = [psh.tile([D, BLK], F32, name=f"h{c}", tag="h")
                      for c in range(4)]
                for i in range(K):
                    for c in range(4):
                        nc.tensor.matmul(
                            hs[c][:],
                            lhsT=wk_s[i][:],
                            rhs=gts[c + i][:],
                            start=(i == 0),
                            stop=(i == K - 1),
                        )

                # ---- aggregate bank: m = self + parallel + sum_c elu(h_c)@Wf ----
                m = psm.tile([D, BLK], F32, name="m", tag="m")
                nc.tensor.matmul(m[:], lhsT=wsb["ws"][:],
                                 rhs=xt_r[:],
                                 start=True, stop=False)
                nc.tensor.matmul(m[:], lhsT=wsb["wp"][:],
                                 rhs=gts[6][:],
                                 start=False, stop=False)

                for c in range(4):
                    v_c = actp.tile([D, BLK], F32R, name="v_c", tag="v")
                    if c < 2:
                        nc.scalar.activation(v_c[:], hs[c][:], AF.Relu,
                                             bias=b3_t[:, :1])
                    else:
                        nc.vector.tensor_scalar(
                            out=v_c[:], in0=hs[c][:], scalar1=b3_t[:, :1],
                            scalar2=0.0, op0=OP.add, op1=OP.max)
                    e_c = actp.tile([D, BLK], F32, name="e_c", tag="e")
                    nc.scalar.activation(e_c[:], hs[c][:], AF.Exp,
                                         bias=b3_t[:, :1])
                    u_c = actp.tile([D, BLK], F32R, name="u_c", tag="u")
                    nc.vector.tensor_scalar(
                        out=u_c[:], in0=e_c[:], scalar1=1.0, scalar2=0.0,
                        op0=OP.subtract, op1=OP.min)
                    nc.tensor.matmul(m[:], lhsT=wsb["wf"][:],
                                     rhs=v_c[:],
                                     start=False, stop=False)
                    nc.tensor.matmul(m[:], lhsT=wsb["wf"][:],
                                     rhs=u_c[:],
                                     start=False, stop=(c == 3))

                # ---- final elu(m + cb) ----
                v_f = actp.tile([D, BLK], F32, name="v_f", tag="v")
                nc.scalar.activation(v_f[:], m[:], AF.Relu, bias=cb_t[:, :1])
                e_f = actp.tile([D, BLK], F32, name="e_f", tag="e")
                nc.scalar.activation(e_f[:], m[:], AF.Exp, bias=cb_t[:, :1])
                u_f = actp.tile([D, BLK], F32, name="u_f", tag="u")
                nc.vector.tensor_scalar(
                    out=u_f[:], in0=e_f[:], scalar1=1.0, scalar2=0.0,
                    op0=OP.subtract, op1=OP.min)
                o_t = outp.tile([D, BLK], F32, name="o_t", tag="o")
                nc.vector.tensor_tensor(out=o_t[:], in0=v_f[:], in1=u_f[:],
                                        op=OP.add)
                nc.sync.dma_start(out=yt[:, b * BLK:(b + 1) * BLK], in_=o_t[:])

    nc.compile()
    return nc


def build_in_maps(inputs: dict, nb: int = NB):
    """Shard/arrange FULL inputs into 8 per-core input maps."""
    x = np.ascontiguousarray(np.asarray(inputs["x"], dtype=np.float32))
    ci = np.asarray(inputs["circle_index"], dtype=np.int32)
    pni = np.asarray(inputs["parallel_node_index"], dtype=np.int32)
    Wk = np.asarray(inputs["Wk"], dtype=np.float32)
    bk = np.asarray(inputs["bk"], dtype=np.float32)
    Wf = np.ascontiguousarray(np.asarray(inputs["Wf"], dtype=np.float32))
    bf = np.asarray(inputs["bf"], dtype=np.float32)
    Ws = np.ascontiguousarray(np.asarray(inputs["Ws"], dtype=np.float32))
    bs = np.asarray(inputs["bs"], dtype=np.float32)
    Wp = np.ascontiguousarray(np.asarray(inputs["Wp"], dtype=np.float32))
    bp = np.asarray(inputs["bp"], dtype=np.float32)

    ns = nb * BLK
    npad = ns * NCORES

    idx_all = np.concatenate([ci, pni[:, None]], axis=1)            # [N, 7]
    idx_all = np.clip(idx_all, 0, N - 1)
    idx_pad = np.zeros((npad, NSLOT), np.int32)
    idx_pad[:N] = idx_all[: min(N, npad)]

    xt = np.zeros((D, npad), np.float32)
    xt[:, :N] = x.T[:, : min(N, npad)]

    b3 = (bk[0] + bk[1] + bk[2]).reshape(D, 1).astype(np.float32)
    cb = (4.0 * bf + bs + bp).reshape(D, 1).astype(np.float32)

    common = {
        "xg": x,
        "wk0": np.ascontiguousarray(Wk[0]),
        "wk1": np.ascontiguousarray(Wk[1]),
        "wk2": np.ascontiguousarray(Wk[2]),
        "wf": Wf, "ws": Ws, "wp": Wp,
        "b3": b3, "cb": cb,
    }
    in_maps = []
    for c in range(NCORES):
        sl = idx_pad[c * ns:(c + 1) * ns]                     # [ns, NSLOT]
        # -> [nb, 128(p), NSLOT(j)*SUB(t)] with slot s = j*SUB + t
        idxc = (sl.reshape(nb, SUB, 128, NSLOT)
                  .transpose(0, 2, 3, 1)
                  .reshape(nb, 128, NSLOT * SUB))
        in_maps.append({
            **common,
            "idxg": np.ascontiguousarray(idxc),
            "xts": np.ascontiguousarray(xt[:, c * ns:(c + 1) * ns]),
        })
    return in_maps


def assemble_output(results, nb: int = NB):
    ns = nb * BLK
    out = np.empty((NCORES * ns, D), np.float32)
    for c in range(NCORES):
        out[c * ns:(c + 1) * ns] = results[c]["yt"].T
    return np.ascontiguousarray(out[:N])


_NC_CACHE = {}


def kernel(**inputs) -> np.ndarray:
    if "nc" not in _NC_CACHE:
        _NC_CACHE["nc"] = build_module()
    nc = _NC_CACHE["nc"]
    in_maps = build_in_maps(inputs)
    res = run_bass_kernel_spmd(nc, in_maps, core_ids=list(range(NCORES)))
    return assemble_output(res.results)
